# revision 36
# baseline (speedup 1.0000x reference)
"""Trainium2 Bass kernel for nn_AtLocPlusCriterion_VO.

loss = exp(-srx)*mean|vo_t - tg_t| + srx + exp(-srq)*mean|vo_q - tg_q| + srq
with vo = calc_vo_logq(pred[:-1], pred[1:]) (relative SE(3) pose, log-quaternion).

Sequence-parallel across 8 NeuronCores (1-row halo per shard). Inputs are
resharded host-side into component-major (SoA) bf16 planes so every on-device
access is contiguous and VectorE runs in 2x mode throughout. Per core: 1956
pairs per SBUF partition, 2 tiles of 978. Row phase (qexp via half-angle Sin
LUT, cos via Sin(pi/2 - x)) with Ln/Exp roots in f32; pair phase (rotation by
two cross products, quaternion product, log map via arctan) in bf16 on
VectorE, ordered rotation-first so the arctan LUT chain overlaps the
translation math. |qv_rel|^2 runs on GpSimd. Cross products are issued as
component-group instructions with negative-stride slab views (no slab
replication copies). Mean-L1 reduces via Abs activations with accum_out on
ScalarE; host sums 8x[128,2].
"""
import os
import numpy as np

N_CORES = 8
T_FULL = 2_000_000
NPAIRS = T_FULL - 1          # 1_999_999
D = 1956                     # pairs per partition per core
C = 978                      # pairs per tile (2 tiles)
NT = 2
R = C + 1                    # rows per tile (halo)
R2 = R + 1                   # padded slab pitch (even)
PPC = 128 * D                # 250_368 pairs per core
PAIRS_PAD = N_CORES * PPC    # 2_002_944
ROWS_PAD = PAIRS_PAD + 1

PL = PPC + 1                 # pred plane length
PT = PPC                     # targ plane length

LN2 = float(np.log(2.0))
LN2SQ2 = float(np.log(2.0 * np.sqrt(2.0)))   # i2n carries 2*sqrt2
PI2 = float(np.pi / 2.0)
SQ2 = float(np.sqrt(2.0))

_BUILT = {}


def _patch_act_tables():
    import concourse.bacc as bacc_mod
    import concourse.hw_specs as hw

    if getattr(bacc_mod, "_vo_tables_patched", False):
        return
    orig = hw.get_activation_tables

    def steered(arch, _orig=orig):
        from concourse import mybir as _mb
        AF = _mb.ActivationFunctionType
        t = {k: set(v) for k, v in _orig(arch).items()}
        # Keep all 24 entries (act_func_set_id indexes the original list);
        # drop ln/exp/arctan from the earlier sets so the table-load pass
        # resolves them to natural_log_exp_and_others / trig_and_small.
        t.get("natural_log", set()).discard(AF.Ln)
        t.get("exp_and_others", set()).discard(AF.Exp)
        t.get("sigmoid_and_others", set()).discard(AF.Arctan)
        return t

    bacc_mod.get_activation_tables = steered
    bacc_mod._vo_tables_patched = True


def _build():
    from concourse import bacc, tile, mybir
    from concourse.ap import AP
    from concourse.bass import _add_dep_helper

    _patch_act_tables()

    f32, bf16 = mybir.dt.float32, mybir.dt.bfloat16
    OP = mybir.AluOpType
    AF = mybir.ActivationFunctionType

    nc = bacc.Bacc("TRN2", target_bir_lowering=False, debug=False,
                   num_devices=N_CORES)
    pred_h = nc.declare_dram_parameter("pred", [6 * PL], bf16, isOutput=False)
    targ_h = nc.declare_dram_parameter("targ", [6 * PT], bf16, isOutput=False)
    out_h = nc.declare_dram_parameter("out", [128, 2], f32, isOutput=True)

    for v in (1e-16, -LN2, LN2SQ2, PI2):
        v = float(v)
        if (f32, v) not in nc.const_aps.aps:
            t = nc.alloc_sbuf_tensor(f"uconst-{v}", [128, 1], f32)
            nc.gpsimd.memset(t.ap(), v)
            nc.const_aps.aps[(f32, v)] = t.ap()
    nc.all_engine_barrier()

    def sb(tile_, off, dims):
        base = tile_[:, :]
        return AP(base.tensor, base.offset + off,
                  [[base.ap.to_list()[0][0], 128]] + dims)

    accs = {}
    groups = {}  # (tile, name) -> list of act instructions

    with tile.TileContext(nc) as tc:

        def mkact(tile_i, group, *args, **kw):
            ins = nc.scalar.activation(*args, **kw)
            if group is not None:
                groups.setdefault((tile_i, group), []).append(ins)
            return ins

        with (
            tc.tile_pool(name="inp", bufs=2) as pin,
            tc.tile_pool(name="rowp", bufs=2) as prow,
            tc.tile_pool(name="scr", bufs=1) as pscr,
            tc.tile_pool(name="accp", bufs=10) as pacc,
        ):
            TT = nc.vector.tensor_tensor
            GT = nc.gpsimd.tensor_tensor
            state = {}

            def dma_tile(t):
                tv = pin.tile([128, 3 * R2], bf16, tag="tv")    # logq comps
                nc.sync.dma_start(
                    sb(tv, 0, [[R2, 3], [1, R]]),
                    AP(pred_h, 3 * PL + t * C, [[D, 128], [PL, 3], [1, R]]))
                tt = pin.tile([128, 3 * R2], bf16, tag="tt")    # t comps
                nc.sync.dma_start(
                    sb(tt, 0, [[R2, 3], [1, R]]),
                    AP(pred_h, t * C, [[D, 128], [PL, 3], [1, R]]))
                gtt = pin.tile([128, 3 * C], bf16, tag="gtt")   # targ t comps
                nc.sync.dma_start(
                    sb(gtt, 0, [[C, 3], [1, C]]),
                    AP(targ_h, t * C, [[D, 128], [PT, 3], [1, C]]))
                gtq = pin.tile([128, 3 * C], bf16, tag="gtq")   # targ q comps
                nc.sync.dma_start(
                    sb(gtq, 0, [[C, 3], [1, C]]),
                    AP(targ_h, 3 * PT + t * C, [[D, 128], [PT, 3], [1, C]]))
                state[t] = (tv, tt, gtt, gtq)

            def row_alloc(t):
                state[(t, 'row')] = (
                    pscr.tile([128, 3 * R2], bf16, tag="sq", name="sq"),
                    pscr.tile([128, R2], bf16, tag="n2a", name="n2a"),
                    pscr.tile([128, R2], bf16, tag="n2", name="n2"),
                    pscr.tile([128, R2], f32, tag="l", name="l"),
                    pscr.tile([128, R2], f32, tag="nh", name="nh"),
                    pscr.tile([128, R2], bf16, tag="i2n", name="i2n"),
                    pscr.tile([128, R2], bf16, tag="sh", name="sh"),
                    pscr.tile([128, R2], bf16, tag="ch", name="ch"),
                    pscr.tile([128, R2], bf16, tag="shsq", name="shsq"),
                    pscr.tile([128, R2], bf16, tag="sinn", name="sinn"),
                    pscr.tile([128, R2], bf16, tag="sn", name="sn"),
                    prow.tile([128, 4 * R2], bf16, tag="AU", name="AU"),
                )

            def row_acts(t, lo=0, hi=R):
                """ScalarE Square (LUT chain head)."""
                tv = state[t][0]
                sq = state[(t, 'row')][0]
                n = hi - lo
                # for tile 0 the Square rides in rowLEb so the chain order is
                # [sq_h1, l_h1, nh_h1, i2n_h1, sq_h2, ...]
                mkact(t, 'rowLEb' if t == 0 else 'rowLEa',
                      sb(sq, lo, [[R2, 3], [1, n]]),
                      sb(tv, lo, [[R2, 3], [1, n]]), AF.Square)

            def row_acts_le(t, lo=0, hi=R):
                l, nh, i2n = state[(t, 'row')][3], state[(t, 'row')][4], \
                    state[(t, 'row')][5]
                mkact(t, 'rowLEb', i2n[:, lo:hi], l[:, lo:hi], AF.Exp,
                      bias=LN2SQ2, scale=-0.5)                   # 2*sqrt2/n

            def row_acts_tr(t, lo=0, hi=R):
                nh, sh, ch, shsq = (state[(t, 'row')][k] for k in (4, 6, 7, 8))
                mkact(t, 'rowTR', sh[:, lo:hi], nh[:, lo:hi], AF.Sin)
                mkact(t, 'rowTR', ch[:, lo:hi], nh[:, lo:hi], AF.Sin,
                      bias=PI2, scale=-1.0)                      # cos(n/2)
                mkact(t, 'rowTR', shsq[:, lo:hi], sh[:, lo:hi], AF.Square)

            def row_vec_a(t, lo=0, hi=R):
                """V: n2 chain + S: l, nh (issued here to sit between sq and
                the sin calls in the act chain)."""
                sq, n2a, n2, l, nh = state[(t, 'row')][0:5]
                TT(n2a[:, lo:hi], sq[:, lo:hi], sq[:, R2 + lo:R2 + hi],
                   OP.add)
                TT(n2[:, lo:hi], n2a[:, lo:hi],
                   sq[:, 2 * R2 + lo:2 * R2 + hi], OP.add)
                mkact(t, 'rowLEb', l[:, lo:hi], n2[:, lo:hi], AF.Ln,
                      bias=1e-16)
                mkact(t, 'rowLEb', nh[:, lo:hi], l[:, lo:hi], AF.Exp,
                      bias=-LN2, scale=0.5)                      # n/2

            def row_vec_b(t, lo=0, hi=R):
                tv = state[t][0]
                (sq, n2a, n2, l, nh, i2n, sh, ch, shsq, sinn, sn, AU) = \
                    state[(t, 'row')]
                n = hi - lo
                TT(sinn[:, lo:hi], sh[:, lo:hi], ch[:, lo:hi], OP.mult)
                # A = sqrt2*cos(n) = sqrt2 - 2*sqrt2*sin^2(n/2)
                mkact(t, 'rowTR', sb(AU, lo, [[1, n]]), shsq[:, lo:hi],
                      AF.Copy, bias=SQ2, scale=-2.0 * SQ2)
                TT(sn[:, lo:hi], sinn[:, lo:hi], i2n[:, lo:hi], OP.mult)
                # U = v * sqrt2*sin(n)/n
                TT(sb(AU, R2 + lo, [[R2, 3], [1, n]]),
                   sb(tv, lo, [[R2, 3], [1, n]]),
                   sb(sn, lo, [[0, 3], [1, n]]), OP.mult)
                state[(t, 'AU')] = AU

            def pair_d(t):
                tt = state[t][1]
                d = pscr.tile([128, 3 * C], bf16, tag="d")
                TT(sb(d, 0, [[C, 3], [1, C]]),
                   sb(tt, 1, [[R2, 3], [1, C]]),
                   sb(tt, 0, [[R2, 3], [1, C]]), OP.subtract)
                state[(t, 'd')] = d

            def cross_into(x12, AUs, v_t, v_row, v_pitch, v_is_AU):
                """x12 slabs 0-2 <- U_{c+1}@r0 * V_{c+2}@v_row,
                slabs 3-5 <- U_{c+2}@r0 * V_{c+1}@v_row  (c = 0,1,2).
                3 instructions: two c-in-{0,1} halves + fused c=2 pair."""
                def V(slab, nsl, sstride):
                    base = (1 + slab) * R2 if v_is_AU else slab * v_pitch
                    ss = sstride * (R2 if v_is_AU else v_pitch)
                    return sb(v_t, base + v_row, [[ss, nsl], [1, C]])
                TT(sb(x12, 0, [[C, 2], [1, C]]),
                   AUs(2, 0, 2), V(2, 2, -2), OP.mult)
                TT(sb(x12, 3 * C, [[C, 2], [1, C]]),
                   AUs(3, 0, 2, -2 * R2), V(1, 2, 1), OP.mult)
                # c = 2 for both halves: (U0*V1 | U1*V0)
                TT(sb(x12, 2 * C, [[3 * C, 2], [1, C]]),
                   AUs(1, 0, 2), V(1, 2, -1), OP.mult)

            def pair_rot(t):
                tv, tt, gtt, gtq = state[t]
                AU = state[(t, 'AU')]
                cm = lambda tl: sb(tl, 0, [[C, 3], [1, C]])

                def AUs(slab, row_off, n_slab, slab_stride=None):
                    ss = R2 if slab_stride is None else slab_stride
                    return sb(AU, slab * R2 + row_off, [[ss, n_slab], [1, C]])

                x12 = pscr.tile([128, 6 * C], bf16, tag="x12")

                # ---- rotation products first: qs2 = 2*qs_rel, qV = 2*qv_rel
                P = pscr.tile([128, 4 * C], bf16, tag="P")
                TT(sb(P, 0, [[C, 4], [1, C]]),
                   sb(AU, 0, [[R2, 4], [1, C]]),
                   sb(AU, 1, [[R2, 4], [1, C]]), OP.mult)
                u = pscr.tile([128, 2 * C], bf16, tag="u")
                TT(sb(u, 0, [[C, 2], [1, C]]),
                   sb(P, 0, [[C, 2], [1, C]]),
                   sb(P, 2 * C, [[C, 2], [1, C]]), OP.add)
                qs2 = pscr.tile([128, C], bf16, tag="qs2")
                TT(qs2[:], u[:, 0:C], u[:, C:2 * C], OP.add)

                # w1 = A0*U1 - A1*U0
                TT(cm(x12), sb(AU, 0, [[0, 3], [1, C]]),
                   sb(AU, R2 + 1, [[R2, 3], [1, C]]), OP.mult)
                TT(sb(x12, 3 * C, [[C, 3], [1, C]]),
                   sb(AU, 1, [[0, 3], [1, C]]),
                   sb(AU, R2, [[R2, 3], [1, C]]), OP.mult)
                w1 = pscr.tile([128, 3 * C], bf16, tag="w1")
                TT(cm(w1), cm(x12), sb(x12, 3 * C, [[C, 3], [1, C]]),
                   OP.subtract)
                # cr = U0 x U1 ; qV = w1 - cr
                cross_into(x12, AUs, AU, 1, None, True)
                qV = pscr.tile([128, 3 * C], bf16, tag="qV")
                TT(cm(qV), cm(x12), sb(x12, 3 * C, [[C, 3], [1, C]]),
                   OP.subtract)                           # qV <- cr temp
                TT(cm(qV), cm(w1), cm(qV), OP.subtract)   # qV = w1 - cr

                # |qV|^2 square on ScalarE (overlaps V translation below;
                # GpSimd would contend with VectorE for the SBUF port)
                qsq = pscr.tile([128, 3 * C], bf16, tag="qsq")
                mkact(t, 'pairLE', cm(qsq), cm(qV), AF.Square)
                state[(t, 'rot')] = (qV, qsq, qs2)

            def pair_trans(t):
                tv, tt, gtt, gtq = state.pop(t)
                AU = state.pop((t, 'AU'))
                d = state.pop((t, 'd'))
                qV, qsq, qs2 = state.pop((t, 'rot'))
                cm = lambda tl: sb(tl, 0, [[C, 3], [1, C]])

                def AUs(slab, row_off, n_slab, slab_stride=None):
                    ss = R2 if slab_stride is None else slab_stride
                    return sb(AU, slab * R2 + row_off, [[ss, n_slab], [1, C]])

                x12 = pscr.tile([128, 6 * C], bf16, tag="x12")

                def cross_into(v_t, v_row, v_pitch, v_is_AU):
                    """x12 slabs 0-2 <- U_{c+1}@r0 * V_{c+2}@v_row,
                    slabs 3-5 <- U_{c+2}@r0 * V_{c+1}@v_row  (c = 0,1,2).
                    3 instructions: the two c-in-{0,1} halves + fused c=2."""
                    def V(slab, nsl, sstride):
                        base = (1 + slab) * R2 if v_is_AU else slab * v_pitch
                        ss = sstride * (R2 if v_is_AU else v_pitch)
                        return sb(v_t, base + v_row, [[ss, nsl], [1, C]])
                    TT(sb(x12, 0, [[C, 2], [1, C]]),
                       AUs(2, 0, 2), V(2, 2, -2), OP.mult)
                    TT(sb(x12, 3 * C, [[C, 2], [1, C]]),
                       AUs(3, 0, 2, slab_stride=-2 * R2), V(1, 2, 1), OP.mult)
                    # c = 2 for both halves: (U0*V1 | U1*V0)
                    TT(sb(x12, 2 * C, [[3 * C, 2], [1, C]]),
                       AUs(1, 0, 2), V(1, 2, -1), OP.mult)

                # ---- translation (overlaps the LUT chain)
                b = pscr.tile([128, 3 * C], bf16, tag="b")
                cp = pscr.tile([128, 3 * C], bf16, tag="cp")
                m = pscr.tile([128, 3 * C], bf16, tag="m")
                g = pscr.tile([128, 3 * C], bf16, tag="g")
                dff = pscr.tile([128, 3 * C], bf16, tag="dff")

                cross_into(d, 0, C, False)
                TT(cm(b), cm(x12), sb(x12, 3 * C, [[C, 3], [1, C]]),
                   OP.subtract)                           # b = U0 x d
                cross_into(b, 0, C, False)
                TT(cm(cp), cm(x12), sb(x12, 3 * C, [[C, 3], [1, C]]),
                   OP.subtract)                           # cp = U0 x b
                # |qV|^2 sums on V here, by which point ScalarE's qsq is done
                nva = pscr.tile([128, C], bf16, tag="nva")
                TT(nva[:], qsq[:, 0:C], qsq[:, C:2 * C], OP.add)
                nv2 = pscr.tile([128, C], bf16, tag="nv2")
                TT(nv2[:], nva[:], qsq[:, 2 * C:3 * C], OP.add)
                lq = pscr.tile([128, C], f32, tag="lq")
                mkact(t, 'pairLE', lq[:], nv2[:], AF.Ln, bias=1e-16)
                rs = pscr.tile([128, C], bf16, tag="rs")
                mkact(t, 'pairLE', rs[:], lq[:], AF.Exp, scale=-0.5)
                TT(cm(m), sb(AU, 0, [[0, 3], [1, C]]), cm(b), OP.mult)
                r2 = pscr.tile([128, C], bf16, tag="r2")
                TT(r2[:], qs2[:], rs[:], OP.mult)
                at = pscr.tile([128, C], bf16, tag="at")
                mkact(t, 'pairTRa', at[:], r2[:], AF.Arctan, scale=-1.0)
                TT(cm(g), cm(d), cm(gtt), OP.subtract)
                TT(cm(g), cm(g), cm(cp), OP.add)
                TT(cm(dff), cm(g), cm(m), OP.subtract)
                acc_t = pacc.tile([128, 1], f32, tag=f"acct{t}")
                dump = pscr.tile([128, 3 * C], bf16, tag="dump")
                mkact(t, 'pairTRb', cm(dump), cm(dff), AF.Abs,
                      accum_out=acc_t[:])
                accs[("t", t)] = acc_t
                state[(t, 'tail')] = (qV, at, rs, gtq, x12, dff, dump)

            def pair_tail(t):
                qV, at, rs, gtq, x12, dff, dump = state.pop((t, 'tail'))
                cm = lambda tl: sb(tl, 0, [[C, 3], [1, C]])
                ratio = pscr.tile([128, C], bf16, tag="ratio")
                # ratio = (at + pi/2) * rs   [theta / (2m)]
                nc.vector.scalar_tensor_tensor(ratio[:], at[:], PI2, rs[:],
                                               OP.add, OP.mult)
                ld = lambda o, n: sb(x12, o, [[C, 3], [1, n]])
                df = lambda o, n: sb(dff, o, [[C, 3], [1, n]])
                gq = lambda o, n: sb(gtq, o, [[C, 3], [1, n]])
                TT(cm(x12), cm(qV), sb(ratio, 0, [[0, 3], [1, C]]), OP.mult)
                acc_q = pacc.tile([128, 1], f32, tag=f"accq{t}")
                if t < NT - 1:
                    TT(cm(dff), cm(x12), cm(gtq), OP.subtract)
                    mkact(t, 'pairTRb', cm(dump), cm(dff), AF.Abs,
                          accum_out=acc_q[:])
                    accs[("q", t)] = (acc_q,)
                else:
                    # split the final |ldiff| so ScalarE overlaps VectorE
                    h = C // 2
                    acc_q2 = pacc.tile([128, 1], f32, tag=f"accq{t}b")
                    TT(df(0, h), ld(0, h), gq(0, h), OP.subtract)
                    mkact(t, 'pairTRb', sb(dump, 0, [[C, 3], [1, h]]),
                          df(0, h), AF.Abs, accum_out=acc_q[:])
                    TT(df(h, C - h), ld(h, C - h), gq(h, C - h), OP.subtract)
                    mkact(t, 'pairTRb', sb(dump, h, [[C, 3], [1, C - h]]),
                          df(h, C - h), AF.Abs, accum_out=acc_q2[:])
                    accs[("q", t)] = (acc_q, acc_q2)

            # ---- schedule: software-pipelined over tiles ----
            # dummy act: hoists the first ACT_TABLE_LOAD off the critical
            # head (runs at t~0 instead of after the first DMA)
            dummy = pacc.tile([128, 1], f32, tag="dummy")
            mkact(-1, 'init', dummy[:], nc.const_aps.aps[(f32, PI2)], AF.Ln)
            dma_tile(0)
            row_alloc(0)
            row_acts(0)
            row_vec_a(0)
            row_acts_le(0)
            pair_d(0)                       # V filler during the act chain
            row_acts_tr(0)
            row_vec_b(0)
            for t in range(NT):
                if t + 1 < NT:
                    dma_tile(t + 1)
                    row_alloc(t + 1)
                    row_acts(t + 1)
                pair_rot(t)
                if t + 1 < NT:
                    row_vec_a(t + 1)
                    row_acts_le(t + 1)
                pair_trans(t)
                if t + 1 < NT:
                    pair_d(t + 1)
                pair_tail(t)
                if t + 1 < NT:
                    row_acts_tr(t + 1)
                    row_vec_b(t + 1)

            # chain LUT activations so same-table-set groups run contiguously
            # across tiles: 6 table loads total for NT=2.
            order = [('init', -1), ('rowLEa', 0), ('rowLEb', 0), ('rowTR', 0)]
            for t in range(NT - 1):
                order += [('rowLEa', t + 1), ('pairLE', t),
                          ('rowLEb', t + 1), ('pairTRa', t),
                          ('rowTR', t + 1), ('pairTRb', t)]
            order += [('pairLE', NT - 1), ('pairTRa', NT - 1),
                      ('pairTRb', NT - 1)]
            seq = []
            for gname, ti in order:
                seq.extend(groups.get((ti, gname), []))
            for i in range(1, len(seq)):
                _add_dep_helper(seq[i].ins, seq[i - 1].ins, False,
                                "act table-set grouping")

            tot = pacc.tile([128, 2], f32, tag="tot")

            def reduce_accs(dst, lst):
                while len(lst) > 2:
                    tmp = pacc.tile([128, 1], f32, tag=f"tmp{id(lst[0])}")
                    nc.vector.tensor_tensor(tmp[:], lst[0][:], lst[1][:],
                                            OP.add)
                    lst = [tmp] + lst[2:]
                nc.vector.tensor_tensor(dst, lst[0][:], lst[1][:], OP.add)

            reduce_accs(tot[:, 0:1], [accs[("t", t_)] for t_ in range(NT)])
            reduce_accs(tot[:, 1:2],
                        [a for t_ in range(NT) for a in accs[("q", t_)]])
            nc.sync.dma_start(out_h[:], tot[:])

    nc.compile()
    return nc


def _get_nc():
    if "nc" not in _BUILT:
        _BUILT["nc"] = _build()
    return _BUILT["nc"]


def run_device(pred, targ, trace=False):
    """pred: (1,T,6) f32, targ: (1,T-1,6) f32 -> (sum|dt|, sum|dq|, exec_ns)"""
    import ml_dtypes
    from concourse.bass_utils import run_bass_kernel_spmd

    bf16 = ml_dtypes.bfloat16
    nc = _get_nc()
    p = np.asarray(pred, dtype=np.float32).reshape(-1, 6)
    g = np.asarray(targ, dtype=np.float32).reshape(-1, 6)
    n_dup = ROWS_PAD - p.shape[0]
    p_pad = np.concatenate([p, np.repeat(p[-1:], n_dup, axis=0)], axis=0)
    g_pad = np.concatenate(
        [g, np.zeros((PAIRS_PAD - g.shape[0], 6), np.float32)], axis=0)
    p_pad = p_pad.astype(bf16)
    g_pad = g_pad.astype(bf16)

    in_maps = []
    for c in range(N_CORES):
        s = c * PPC
        in_maps.append({
            "pred": np.ascontiguousarray(p_pad[s:s + PPC + 1].T).reshape(-1),
            "targ": np.ascontiguousarray(g_pad[s:s + PPC].T).reshape(-1),
        })
    res = run_bass_kernel_spmd(nc, in_maps, core_ids=list(range(N_CORES)),
                               trace=trace)
    psum = np.stack([np.asarray(res.results[i]["out"], dtype=np.float64)
                     for i in range(N_CORES)])
    st = float(psum[:, :, 0].sum())
    sq = float(psum[:, :, 1].sum())
    return st, sq, res.exec_time_ns


def kernel(pred, targ, srx, srq):
    trace = bool(int(os.environ.get("VO_KERNEL_TRACE", "0")))
    st, sq, _ = run_device(pred, targ, trace=trace)
    t_loss = st / (3.0 * NPAIRS)
    q_loss = sq / (3.0 * NPAIRS)
    srx_v = float(np.asarray(srx).reshape(-1)[0])
    srq_v = float(np.asarray(srq).reshape(-1)[0])
    out = (np.exp(-srx_v) * t_loss + srx_v +
           np.exp(-srq_v) * q_loss + srq_v)
    return np.array([out], dtype=np.float32)
